# revision 29
# baseline (speedup 1.0000x reference)
"""GRPE sparse-attention TRN2 kernel v2: host prep + Bass program builder.

Per batch b, head h (N=256, D=768, H=12, RD=64, TE=32, TP=40, TC=72):
  q = (x@Wq + bq)*S ; k = x@Wk + bk ; v = x@Wv + bv
  A[i,j] = q_i.k_j + Tq[i,te(i,j)] + Tq[i,32+tp(i,j)] + Tk[j,te] + Tk[j,32+tp]
  E = exp(A); P = E / rowsum(E)
  z = P@v + pool_by_type contributions; y = z@Wo + bo

Type-indexed gathers/pools are dense matmuls against host-built fp8 one-hots:
  OA[b,t,i,j] (q-side mov), OB[b,t,j,i] (k-side mov), OC[b,j,i,t] (pool mov).

v2 vs v1: consolidated remap DMAs (one rearrange-DMA per chunk instead of 48
scatter DMAs), per-batch tile double-buffering + full python unroll for
cross-batch overlap, per-head softmax pipeline.

v3 vs v2: h-major layouts for the score tables (col = h*N + i) and the P^T
tiles tET, so the big PSUM->SBUF evacuation copies and the AV matmul moving
operands are contiguous; the bias/pool matmul stationaries become junk-free
12-column stride-N views (cheaper LDWEIGHTS, 12-row psum blocks).
"""
import sys
import numpy as np
import ml_dtypes

sys.path.insert(0, "/opt/trn_rl_repo")
sys.path.insert(0, "/opt/trn_rl_repo/concourse")

from contextlib import ExitStack
from concourse import bass, bacc, mybir

dt = mybir.dt
AF = mybir.ActivationFunctionType
ALU = mybir.AluOpType

N, D, H, RD = 256, 768, 12, 64
TE, TP, TC = 32, 40, 72
S = RD ** -0.5

IC = 32           # i-streaming chunk for bias/pool matmul phases
NCH = N // IC     # 8 chunks
bf = ml_dtypes.bfloat16
f8 = ml_dtypes.float8_e4m3
ONE_F8 = np.uint8(0x38)


# ---------------------------------------------------------------- host prep
def prep_weights(inp):
    w = {}
    for nm in ("Wq", "Wk", "Wv", "Wo"):
        w[nm] = np.ascontiguousarray(inp[nm]).astype(bf)
    w["bqs"] = np.asarray(inp["bq"], np.float32).reshape(D) * S
    w["bk"] = np.asarray(inp["bk"], np.float32).reshape(D)
    w["bvb"] = np.asarray(inp["bv"], np.float32).astype(bf).reshape(1, D)
    w["bob"] = np.asarray(inp["bo"], np.float32).astype(bf).reshape(1, D)
    eq = np.concatenate([np.transpose(inp["Eeq"], (1, 2, 0)),
                         np.transpose(inp["Epq"], (1, 2, 0))], axis=2)
    ek = np.concatenate([np.transpose(inp["Eek"], (1, 2, 0)),
                         np.transpose(inp["Epk"], (1, 2, 0))], axis=2) * S
    w["Eqcat"] = np.ascontiguousarray(eq).astype(bf)   # [H, 64, 72]
    w["Ekcat"] = np.ascontiguousarray(ek).astype(bf)   # [H, 64, 72]
    wc = np.concatenate([np.transpose(inp["Eev"], (1, 0, 2)),
                         np.transpose(inp["Epv"], (1, 0, 2))], axis=1)
    w["Wcat"] = np.ascontiguousarray(wc).astype(bf)    # [H, 72, 64]
    w["ones_row"] = np.ones((1, 128), dtype=bf)
    return w


def prep_shard(node_reps, conn, dist, b0, nb):
    sl = slice(b0, b0 + nb)
    x = np.asarray(node_reps[sl], np.float32)
    te = np.asarray(conn[sl], np.int64)
    tp = np.asarray(dist[sl], np.int64)
    d = {}
    d["xT"] = np.ascontiguousarray(np.transpose(x, (0, 2, 1))).astype(bf)

    bidx = np.arange(nb)[:, None, None]
    ii = np.arange(N)[None, :, None]
    jj = np.arange(N)[None, None, :]

    oa = np.zeros((nb, TC, N, N), dtype=np.uint8)
    oa[bidx, te, ii, jj] = ONE_F8
    oa[bidx, TE + tp, ii, jj] = ONE_F8
    d["OA"] = oa.view(f8)

    teT = np.ascontiguousarray(np.transpose(te, (0, 2, 1)))
    tpT = np.ascontiguousarray(np.transpose(tp, (0, 2, 1)))
    ob = np.zeros((nb, TC, N, N), dtype=np.uint8)
    ob[bidx, teT, ii, jj] = ONE_F8
    ob[bidx, TE + tpT, ii, jj] = ONE_F8
    d["OB"] = ob.view(f8)

    oc = np.zeros((nb, N, N, TC), dtype=np.uint8)
    iiT = np.arange(N)[None, None, :]
    jjT = np.arange(N)[None, :, None]
    oc[bidx, jjT, iiT, teT] = ONE_F8
    oc[bidx, jjT, iiT, TE + tpT] = ONE_F8
    d["OC"] = oc.view(f8)
    return d


# ------------------------------------------------------------- program build
def build_program(nb, num_devices=8, use_for_i=False):
    nc = bacc.Bacc("TRN2", target_bir_lowering=False, debug=False,
                   num_devices=num_devices, detect_race_conditions=False)

    def din(name, shape, dty):
        return nc.dram_tensor(name, list(shape), dty, kind="ExternalInput").ap()

    xT_d = din("xT", (nb, D, N), dt.bfloat16)
    OA_d = din("OA", (nb, TC, N, N), dt.float8e4)
    OB_d = din("OB", (nb, TC, N, N), dt.float8e4)
    OC_d = din("OC", (nb, N, N, TC), dt.float8e4)
    Wq_d = din("Wq", (D, D), dt.bfloat16)
    Wk_d = din("Wk", (D, D), dt.bfloat16)
    Wv_d = din("Wv", (D, D), dt.bfloat16)
    Wo_d = din("Wo", (D, D), dt.bfloat16)
    bqs_d = din("bqs", (D,), dt.float32)
    bk_d = din("bk", (D,), dt.float32)
    bvb_d = din("bvb", (1, D), dt.bfloat16)
    bob_d = din("bob", (1, D), dt.bfloat16)
    Eq_d = din("Eqcat", (H, RD, TC), dt.bfloat16)
    Ek_d = din("Ekcat", (H, RD, TC), dt.bfloat16)
    Wc_d = din("Wcat", (H, TC, RD), dt.bfloat16)
    ones_d = din("ones_row", (1, 128), dt.bfloat16)
    y_d = nc.dram_tensor("y", [nb, N, D], dt.float32, kind="ExternalOutput").ap()

    from concourse.tile import TileContext

    with TileContext(nc) as tc, ExitStack() as ctx:
        const = ctx.enter_context(tc.tile_pool(name="const", bufs=1))
        pers = ctx.enter_context(tc.tile_pool(name="pers", bufs=1))
        dbl = ctx.enter_context(tc.tile_pool(name="dbl", bufs=2))
        ohp = ctx.enter_context(tc.tile_pool(name="ohp", bufs=2))
        ohc = ctx.enter_context(tc.tile_pool(name="ohc", bufs=3))
        stpp = ctx.enter_context(tc.tile_pool(name="stpp", bufs=3))
        pp = ctx.enter_context(tc.tile_pool(name="pp", bufs=5, space="PSUM"))
        pt = ctx.enter_context(tc.tile_pool(name="pt", bufs=2, space="PSUM"))
        pz = ctx.enter_context(tc.tile_pool(name="pz", bufs=1, space="PSUM"))

        # ---- persistent constants ----
        tWq = const.tile([128, 6 * D], dt.bfloat16)
        tWk = const.tile([128, 6 * D], dt.bfloat16)
        tWv = const.tile([128, 6 * D], dt.bfloat16)
        tWo = const.tile([128, 6 * D], dt.bfloat16)
        for t, d_ in ((tWq, Wq_d), (tWk, Wk_d), (tWv, Wv_d), (tWo, Wo_d)):
            nc.sync.dma_start(t[:].rearrange("p (c o) -> p c o", c=6),
                              d_.rearrange("(c p) o -> p c o", p=128))
        tbqs = const.tile([128, 6], dt.float32)
        nc.sync.dma_start(tbqs[:], bqs_d.rearrange("(c p) -> p c", p=128))
        tbk = const.tile([128, 6], dt.float32)
        nc.sync.dma_start(tbk[:], bk_d.rearrange("(c p) -> p c", p=128))
        tbvb = const.tile([1, D], dt.bfloat16); nc.sync.dma_start(tbvb[:], bvb_d[:])
        tbob = const.tile([1, D], dt.bfloat16); nc.sync.dma_start(tbob[:], bob_d[:])
        tEq = const.tile([128, H * TC], dt.bfloat16)
        tEk = const.tile([128, H * TC], dt.bfloat16)
        for base in (0, 64):
            nc.sync.dma_start(tEq[base:base + 64, :].rearrange("d (h t) -> d h t", h=H),
                              Eq_d.rearrange("h d t -> d h t"))
            nc.sync.dma_start(tEk[base:base + 64, :].rearrange("d (h t) -> d h t", h=H),
                              Ek_d.rearrange("h d t -> d h t"))
        tWc = const.tile([TC, H * RD], dt.bfloat16)
        nc.sync.dma_start(tWc[:].rearrange("t (h d) -> t h d", h=H),
                          Wc_d.rearrange("h t d -> t h d"))
        tones = const.tile([1, 128], dt.bfloat16)
        nc.sync.dma_start(tones[:], ones_d[:])
        onesq = const.tile([128, 128], dt.bfloat16)
        nc.vector.memset(onesq[:], 1.0)
        ident = const.tile([128, 128], dt.bfloat16)
        nc.gpsimd.affine_select(ident[:], onesq[:], [[1, 128]], ALU.is_equal,
                                0.0, base=0, channel_multiplier=-1)
        identf = const.tile([128, 128], dt.float32)
        nc.vector.tensor_copy(identf[:], ident[:])

        # ---- persistent (single-buffered) per-batch workspace ----
        tET = [pers.tile([128, H * N], dt.bfloat16, name=f"tET{c_}") for c_ in range(2)]
        tAeT = pers.tile([TC, H * N], dt.bfloat16)
        tZ = pers.tile([128, 2 * H], dt.float32)
        tZr = pers.tile([128, 2 * H], dt.float32)
        tzT = pers.tile([128, 6 * N], dt.bfloat16)
        ty = pers.tile([128, D], dt.float32)
        tBqT = [pers.tile([128, H * N], dt.float32, name=f"tBqT{c_}")
                for c_ in range(2)]

        def qslab(t, h):
            # [64, 256] head-h slab of a [dout-part, (chunk, tok)] projection
            return t[(h % 2) * 64:(h % 2) * 64 + 64, (h // 2) * N:(h // 2) * N + N]

        def body(bi):
            # ---------- load xT ----------
            txT = dbl.tile([128, 6 * N], dt.bfloat16, tag="txT")
            nc.sync.dma_start(txT[:].rearrange("p (c n) -> p c n", c=6),
                              xT_d[bi].rearrange("(c p) n -> p c n", p=128))

            # ---------- projections ----------
            tq = dbl.tile([128, 6 * N], dt.bfloat16, tag="tq")
            tk = dbl.tile([128, 6 * N], dt.bfloat16, tag="tk")
            tv = dbl.tile([128, 2 * D], dt.bfloat16, tag="tv")
            for dc in range(6):
                for tW, tdst, scale, tbias in ((tWq, tq, S, tbqs), (tWk, tk, 1.0, tbk)):
                    ps = pp.tile([128, 512], dt.float32, tag="ps", name="ps_qk")
                    for dn in range(6):
                        nc.tensor.matmul(ps[:, 0:N],
                                         tW[:, dn * D + dc * 128: dn * D + dc * 128 + 128],
                                         txT[:, dn * N: dn * N + N],
                                         start=(dn == 0), stop=(dn == 5))
                    nc.scalar.activation(tdst[:, dc * N:dc * N + N], ps[:, 0:N], AF.Identity,
                                         bias=tbias[:, dc:dc + 1], scale=scale)
            for tcn in range(2):
                for half in range(2):
                    ps = pp.tile([128, 512], dt.float32, tag="ps", name="ps_v")
                    for dn in range(6):
                        nc.tensor.matmul(ps[:, 0:384],
                                         txT[:, dn * N + tcn * 128: dn * N + tcn * 128 + 128],
                                         tWv[:, dn * D + half * 384: dn * D + half * 384 + 384],
                                         start=(dn == 0), stop=False)
                    nc.tensor.matmul(ps[:, 0:384], tones[:], tbvb[:, half * 384: half * 384 + 384],
                                     start=False, stop=True)
                    nc.vector.tensor_copy(tv[:, tcn * D + half * 384: tcn * D + half * 384 + 384],
                                          ps[:, 0:384])

            # ---------- score tables (h-major: col = h*N + i) ----------
            tTq = dbl.tile([TC, H * N], dt.bfloat16, tag="tTq")
            tTk = dbl.tile([TC, H * N], dt.bfloat16, tag="tTk")
            for h in range(H):
                base = (h % 2) * 64
                for tE_, tsrc, tdst in ((tEq, tq, tTq), (tEk, tk, tTk)):
                    ps = pp.tile([128, 512], dt.float32, tag="ps", name="ps_tab")
                    nc.tensor.matmul(ps[0:TC, 0:N],
                                     tE_[base:base + 64, h * TC: h * TC + TC],
                                     qslab(tsrc, h),
                                     start=True, stop=True)
                    nc.scalar.activation(tdst[:, h * N: h * N + N],
                                         ps[0:TC, 0:N], AF.Copy)

            # ---------- bias matmuls (q-side then k-side) ----------
            # Both sides stage [ (c,h), (g,*) ] then PE-transpose out of the
            # awkward orientation. k-side -> tBk[i, (h,j)] bf16 (ident-add);
            # q-side -> tBqT[j, (h,i)] f32 (is_transpose-add into psA).
            tBk = [dbl.tile([128, H * N], dt.bfloat16, tag=f"tBk{c_}", name=f"tBk{c_}")
                   for c_ in range(2)]
            for side in range(2):
                O_d, tT = (OA_d, tTq) if side == 0 else (OB_d, tTk)

                def bias_mm(ch, O_d=O_d, tT=tT):
                    i0 = ch * IC
                    tO = ohp.tile([TC, IC * N], dt.float8e4, tag="oh", name="tO")
                    nc.sync.dma_start(tO[:].rearrange("t (i j) -> t i j", i=IC),
                                      O_d[bi, :, i0:i0 + IC, :])
                    stg = dbl.tile([128, 8 * N], dt.bfloat16, tag="stq")
                    pbufs = [pp.tile([128, 512], dt.float32, tag="ps", name=f"psb{q_}")
                             for q_ in range(2)]
                    tTv = tT[:].rearrange("t (h n) -> t n h", n=N)
                    for g in range(8):
                        ps = pbufs[(g // 2) % 2]
                        col = (g % 2) * N
                        for c in range(4):
                            irel = c * 8 + g
                            nc.tensor.matmul(
                                ps[32 * c:32 * c + 12, col:col + N],
                                tTv[:, i0 + irel, :],
                                tO[:, irel * N: irel * N + N],
                                start=True, stop=True, tile_position=(0, 32 * c))
                        if g % 2 == 1:
                            if (g // 2) % 2 == 0:
                                nc.scalar.activation(stg[:, (g - 1) * N:(g + 1) * N], ps[:], AF.Copy)
                            else:
                                nc.vector.tensor_copy(stg[:, (g - 1) * N:(g + 1) * N], ps[:])
                    return stg

                # PE-transpose [ (c,h), (g,x) ] -> [ x, (c,h) ] and
                # assemble the bias buffer with strided copies.
                # side 0: x=j (chunk over i), dst tBqT[j, (h, i)] f32.
                # side 1: x=i (chunk over j), dst tBk[i, (h, j)] bf16.
                def bias_tr(ch, stg, side=side):
                    for xhalf in range(2):
                        if side == 0:
                            dstv = tBqT[xhalf][:].rearrange(
                                "p (h ich c g) -> p ich g c h",
                                h=H, ich=NCH, c=4)
                        else:
                            dstv = tBk[xhalf][:].rearrange(
                                "p (h jch c g) -> p jch g c h",
                                h=H, jch=NCH, c=4)
                        for gb in range(2):
                            pst = pt.tile([128, 512], dt.bfloat16,
                                          tag="pstr", name="pstb")
                            for gg in range(4):
                                g = gb * 4 + gg
                                nc.tensor.matmul(
                                    pst[:, gg * 128: gg * 128 + 128],
                                    stg[:, g * N + xhalf * 128:
                                        g * N + xhalf * 128 + 128],
                                    ident[:], is_transpose=True,
                                    start=True, stop=True)
                            srcv = pst[:].rearrange(
                                "p (gg c h2) -> p gg c h2", gg=4, c=4)
                            if (gb + xhalf) % 2 == 0:
                                nc.vector.tensor_copy(
                                    dstv[:, ch, gb * 4:(gb + 1) * 4],
                                    srcv[:, :, :, 0:H])
                            else:
                                nc.scalar.activation(
                                    dstv[:, ch, gb * 4:(gb + 1) * 4],
                                    srcv[:, :, :, 0:H], AF.Copy)

                prev_stg = None
                for ch in range(NCH):
                    stg_c = bias_mm(ch)
                    if prev_stg is not None:
                        bias_tr(ch - 1, prev_stg)
                    prev_stg = stg_c
                bias_tr(NCH - 1, prev_stg)

            # ---------- attention + per-head softmax/transpose ----------
            for h in range(H):
                psA = pp.tile([128, 512], dt.float32, tag="ps", name="psA")
                tEa = dbl.tile([128, 512], dt.bfloat16, tag="tEa")
                for icx in range(2):
                    acol = icx * N
                    nc.tensor.matmul(psA[:, acol:acol + N],
                                     qslab(tq, h)[:, icx * 128: icx * 128 + 128],
                                     qslab(tk, h),
                                     start=True, stop=False)
                    nc.tensor.matmul(psA[:, acol:acol + N], ident[:],
                                     tBk[icx][:, h * N: h * N + N],
                                     start=False, stop=False)
                    for jc in range(2):
                        nc.tensor.matmul(
                            psA[:, acol + jc * 128: acol + jc * 128 + 128],
                            tBqT[jc][:, h * N + icx * 128: h * N + icx * 128 + 128],
                            identf[:],
                            is_transpose=True, start=False, stop=(jc == 1))
                    nc.scalar.activation(
                        tEa[:, acol:acol + N],
                        psA[:, acol:acol + N], AF.Exp,
                        accum_out=tZ[:, 2 * h + icx: 2 * h + icx + 1])
                nc.vector.reciprocal(tZr[:, 2 * h:2 * h + 2], tZ[:, 2 * h:2 * h + 2])
                for icx in range(2):
                    nc.vector.tensor_scalar(
                        tEa[:, icx * N:icx * N + N],
                        tEa[:, icx * N:icx * N + N],
                        tZr[:, 2 * h + icx: 2 * h + icx + 1], None, ALU.mult)
                for jc in range(2):
                    pst = pt.tile([128, 512], dt.bfloat16, tag="pstr", name="pst")
                    for icx in range(2):
                        nc.tensor.matmul(pst[:, icx * 128: icx * 128 + 128],
                                         tEa[:, icx * N + jc * 128:
                                             icx * N + jc * 128 + 128],
                                         ident[:], is_transpose=True,
                                         start=True, stop=True)
                    nc.vector.tensor_copy(tET[jc][:, h * N: h * N + N],
                                          pst[:, 0:N])

            # ---------- pooling matmuls ----------
            def pool_mm(ch):
                i0 = ch * IC
                tOC = [ohc.tile([128, IC * TC], dt.float8e4, tag="ohc", name=f"tOC{jc}")
                       for jc in range(2)]
                for jc in range(2):
                    nc.sync.dma_start(tOC[jc][:].rearrange("j (i t) -> j i t", i=IC),
                                      OC_d[bi, jc * 128:jc * 128 + 128, i0:i0 + IC, :])
                stp = stpp.tile([128, 8 * TC], dt.bfloat16, tag="stp")
                pbufs = [pp.tile([128, 512], dt.float32, tag="ps", name=f"psp{q_}")
                         for q_ in range(2)]
                tETv = [tET[jc][:].rearrange("j (h n) -> j n h", n=N)
                        for jc in range(2)]
                for g in range(8):
                    ps = pbufs[(g // 4) % 2]
                    col = (g % 4) * TC
                    for c in range(4):
                        irel = c * 8 + g
                        for jc in range(2):
                            nc.tensor.matmul(
                                ps[32 * c:32 * c + 12, col:col + TC],
                                tETv[jc][:, i0 + irel, :],
                                tOC[jc][:, irel * TC: irel * TC + TC],
                                start=(jc == 0), stop=(jc == 1),
                                tile_position=(0, 32 * c))
                    if g % 4 == 3:
                        if (g // 4) % 2 == 0:
                            nc.scalar.activation(stp[:, (g - 3) * TC:(g + 1) * TC],
                                                 ps[:, 0:4 * TC], AF.Copy)
                        else:
                            nc.vector.tensor_copy(stp[:, (g - 3) * TC:(g + 1) * TC],
                                                  ps[:, 0:4 * TC])
                return stp

            # PE-transpose [ (c,h), (g,t) ] -> [ t, (c,h) ] and assemble
            # tAeT[t, (h, i0+c*8+g)] with strided copies.
            def pool_tr(ch, stp):
                dstpv = tAeT[:].rearrange("t (h ich c g) -> t ich g c h",
                                          h=H, ich=NCH, c=4)
                for gb in range(2):
                    pst = pt.tile([128, 512], dt.bfloat16, tag="pstr", name="pstp")
                    for gg in range(4):
                        g = gb * 4 + gg
                        nc.tensor.matmul(pst[0:TC, gg * 128: gg * 128 + 128],
                                         stp[:, g * TC: g * TC + TC],
                                         ident[:], is_transpose=True,
                                         start=True, stop=True)
                    srcpv = pst[0:TC, :].rearrange("t (gg c h2) -> t gg c h2",
                                                   gg=4, c=4)
                    if gb == 0:
                        nc.vector.tensor_copy(dstpv[:, ch, gb * 4:(gb + 1) * 4],
                                              srcpv[:, :, :, 0:H])
                    else:
                        nc.scalar.activation(dstpv[:, ch, gb * 4:(gb + 1) * 4],
                                             srcpv[:, :, :, 0:H], AF.Copy)

            prev_stp = None
            for ch in range(NCH):
                stp_c = pool_mm(ch)
                if prev_stp is not None:
                    pool_tr(ch - 1, prev_stp)
                prev_stp = stp_c
            pool_tr(NCH - 1, prev_stp)

            # ---------- AV + pooled values -> z^T ----------
            for h in range(H):
                pzt = pz.tile([128, 512], dt.float32, tag="pz", name="pz")
                for jc in range(2):
                    nc.tensor.matmul(pzt[0:64, 0:N],
                                     tv[:, jc * D + h * RD: jc * D + h * RD + RD],
                                     tET[jc][:, h * N: h * N + N],
                                     start=(jc == 0), stop=False)
                nc.tensor.matmul(pzt[0:64, 0:N], tWc[:, h * RD: h * RD + RD],
                                 tAeT[:, h * N: h * N + N],
                                 start=False, stop=True)
                nc.scalar.activation(qslab(tzT, h), pzt[0:64, 0:N], AF.Copy)

            # ---------- output projection ----------
            for icx in range(2):
                for half in range(2):
                    ps = pp.tile([128, 512], dt.float32, tag="ps", name="psy")
                    for dzc in range(6):
                        nc.tensor.matmul(ps[:, 0:384],
                                         tzT[:, dzc * N + icx * 128: dzc * N + icx * 128 + 128],
                                         tWo[:, dzc * D + half * 384: dzc * D + half * 384 + 384],
                                         start=(dzc == 0), stop=False)
                    nc.tensor.matmul(ps[:, 0:384], tones[:], tbob[:, half * 384: half * 384 + 384],
                                     start=False, stop=True)
                    nc.vector.tensor_copy(ty[:, half * 384: half * 384 + 384], ps[:, 0:384])
                nc.sync.dma_start(y_d[bi][icx * 128: icx * 128 + 128, :], ty[:])

        for b in range(nb):
            body(b)

    nc.compile()
    return nc


# ---------------------------------------------------------------- entry point
_PROGRAM_CACHE = {}


def _get_program(nb, ncores):
    key = (nb, ncores)
    if key not in _PROGRAM_CACHE:
        _PROGRAM_CACHE[key] = build_program(nb, num_devices=ncores)
    return _PROGRAM_CACHE[key]


def kernel(node_reps, connection_reps, distance, mask,
           Wq, bq, Wk, bk, Wv, bv, Wo, bo,
           Eeq, Eek, Eev, Epq, Epk, Epv):
    """Full-input GRPE attention on 8 TRN2 NeuronCores (data-parallel over batch)."""
    import antenv
    if '/opt/trn_rl_repo/antenv' not in antenv.__path__:
        antenv.__path__.append('/opt/trn_rl_repo/antenv')
    try:
        import antenv.axon_hooks as axon_hooks
        axon_hooks.register_default_hook()
    except Exception:
        pass
    from concourse.bass_utils import run_bass_kernel_spmd

    node_reps = np.asarray(node_reps)
    connection_reps = np.asarray(connection_reps)
    distance = np.asarray(distance)
    B = node_reps.shape[0]
    NCORES = 8
    assert B % NCORES == 0
    nb = B // NCORES

    inp = dict(Wq=np.asarray(Wq), bq=np.asarray(bq), Wk=np.asarray(Wk),
               bk=np.asarray(bk), Wv=np.asarray(Wv), bv=np.asarray(bv),
               Wo=np.asarray(Wo), bo=np.asarray(bo),
               Eeq=np.asarray(Eeq), Eek=np.asarray(Eek), Eev=np.asarray(Eev),
               Epq=np.asarray(Epq), Epk=np.asarray(Epk), Epv=np.asarray(Epv))
    w = prep_weights(inp)
    shards = [prep_shard(node_reps, connection_reps, distance, c * nb, nb)
              for c in range(NCORES)]

    nc = _get_program(nb, NCORES)
    in_maps = [{**w, **shards[c]} for c in range(NCORES)]
    res = run_bass_kernel_spmd(nc, in_maps, list(range(NCORES)))
    out = np.concatenate([res.results[c]["y"] for c in range(NCORES)], axis=0)
    return out.astype(np.float32)



# revision 33
# speedup vs baseline: 1.0387x; 1.0387x over previous
"""GRPE sparse-attention TRN2 kernel v2: host prep + Bass program builder.

Per batch b, head h (N=256, D=768, H=12, RD=64, TE=32, TP=40, TC=72):
  q = (x@Wq + bq)*S ; k = x@Wk + bk ; v = x@Wv + bv
  A[i,j] = q_i.k_j + Tq[i,te(i,j)] + Tq[i,32+tp(i,j)] + Tk[j,te] + Tk[j,32+tp]
  E = exp(A); P = E / rowsum(E)
  z = P@v + pool_by_type contributions; y = z@Wo + bo

Type-indexed gathers/pools are dense matmuls against host-built fp8 one-hots:
  OA[b,t,i,j] (q-side mov), OB[b,t,j,i] (k-side mov), OC[b,j,i,t] (pool mov).

v2 vs v1: consolidated remap DMAs (one rearrange-DMA per chunk instead of 48
scatter DMAs), per-batch tile double-buffering + full python unroll for
cross-batch overlap, per-head softmax pipeline.

v3 vs v2: h-major layouts for the score tables (col = h*N + i) and the P^T
tiles tET, so the big PSUM->SBUF evacuation copies and the AV matmul moving
operands are contiguous; the bias/pool matmul stationaries become junk-free
12-column stride-N views (cheaper LDWEIGHTS, 12-row psum blocks).
"""
import sys
import numpy as np
import ml_dtypes

sys.path.insert(0, "/opt/trn_rl_repo")
sys.path.insert(0, "/opt/trn_rl_repo/concourse")

from contextlib import ExitStack
from concourse import bass, bacc, mybir

dt = mybir.dt
AF = mybir.ActivationFunctionType
ALU = mybir.AluOpType

N, D, H, RD = 256, 768, 12, 64
TE, TP, TC = 32, 40, 72
S = RD ** -0.5

IC = 32           # i-streaming chunk for bias/pool matmul phases
NCH = N // IC     # 8 chunks
bf = ml_dtypes.bfloat16
f8 = ml_dtypes.float8_e4m3
ONE_F8 = np.uint8(0x38)


# ---------------------------------------------------------------- host prep
def prep_weights(inp):
    w = {}
    for nm in ("Wq", "Wk", "Wv", "Wo"):
        w[nm] = np.ascontiguousarray(inp[nm]).astype(bf)
    w["bqs"] = np.asarray(inp["bq"], np.float32).reshape(D) * S
    w["bk"] = np.asarray(inp["bk"], np.float32).reshape(D)
    w["bvb"] = np.asarray(inp["bv"], np.float32).astype(bf).reshape(1, D)
    w["bob"] = np.asarray(inp["bo"], np.float32).astype(bf).reshape(1, D)
    eq = np.concatenate([np.transpose(inp["Eeq"], (1, 2, 0)),
                         np.transpose(inp["Epq"], (1, 2, 0))], axis=2)
    ek = np.concatenate([np.transpose(inp["Eek"], (1, 2, 0)),
                         np.transpose(inp["Epk"], (1, 2, 0))], axis=2) * S
    w["Eqcat"] = np.ascontiguousarray(eq).astype(bf)   # [H, 64, 72]
    w["Ekcat"] = np.ascontiguousarray(ek).astype(bf)   # [H, 64, 72]
    wc = np.concatenate([np.transpose(inp["Eev"], (1, 0, 2)),
                         np.transpose(inp["Epv"], (1, 0, 2))], axis=1)
    w["Wcat"] = np.ascontiguousarray(wc).astype(bf)    # [H, 72, 64]
    w["ones_row"] = np.ones((1, 128), dtype=bf)
    return w


def prep_shard(node_reps, conn, dist, b0, nb):
    sl = slice(b0, b0 + nb)
    x = np.asarray(node_reps[sl], np.float32)
    te = np.asarray(conn[sl], np.int64)
    tp = np.asarray(dist[sl], np.int64)
    d = {}
    d["xT"] = np.ascontiguousarray(np.transpose(x, (0, 2, 1))).astype(bf)

    bidx = np.arange(nb)[:, None, None]
    ii = np.arange(N)[None, :, None]
    jj = np.arange(N)[None, None, :]

    oa = np.zeros((nb, TC, N, N), dtype=np.uint8)
    oa[bidx, te, ii, jj] = ONE_F8
    oa[bidx, TE + tp, ii, jj] = ONE_F8
    d["OA"] = oa.view(f8)

    teT = np.ascontiguousarray(np.transpose(te, (0, 2, 1)))
    tpT = np.ascontiguousarray(np.transpose(tp, (0, 2, 1)))
    ob = np.zeros((nb, TC, N, N), dtype=np.uint8)
    ob[bidx, teT, ii, jj] = ONE_F8
    ob[bidx, TE + tpT, ii, jj] = ONE_F8
    d["OB"] = ob.view(f8)

    oc = np.zeros((nb, N, N, TC), dtype=np.uint8)
    iiT = np.arange(N)[None, None, :]
    jjT = np.arange(N)[None, :, None]
    oc[bidx, jjT, iiT, teT] = ONE_F8
    oc[bidx, jjT, iiT, TE + tpT] = ONE_F8
    d["OC"] = oc.view(f8)
    return d


# ------------------------------------------------------------- program build
def build_program(nb, num_devices=8, use_for_i=False):
    nc = bacc.Bacc("TRN2", target_bir_lowering=False, debug=False,
                   num_devices=num_devices, detect_race_conditions=False)

    def din(name, shape, dty):
        return nc.dram_tensor(name, list(shape), dty, kind="ExternalInput").ap()

    xT_d = din("xT", (nb, D, N), dt.bfloat16)
    OA_d = din("OA", (nb, TC, N, N), dt.float8e4)
    OB_d = din("OB", (nb, TC, N, N), dt.float8e4)
    OC_d = din("OC", (nb, N, N, TC), dt.float8e4)
    Wq_d = din("Wq", (D, D), dt.bfloat16)
    Wk_d = din("Wk", (D, D), dt.bfloat16)
    Wv_d = din("Wv", (D, D), dt.bfloat16)
    Wo_d = din("Wo", (D, D), dt.bfloat16)
    bqs_d = din("bqs", (D,), dt.float32)
    bk_d = din("bk", (D,), dt.float32)
    bvb_d = din("bvb", (1, D), dt.bfloat16)
    bob_d = din("bob", (1, D), dt.bfloat16)
    Eq_d = din("Eqcat", (H, RD, TC), dt.bfloat16)
    Ek_d = din("Ekcat", (H, RD, TC), dt.bfloat16)
    Wc_d = din("Wcat", (H, TC, RD), dt.bfloat16)
    ones_d = din("ones_row", (1, 128), dt.bfloat16)
    y_d = nc.dram_tensor("y", [nb, N, D], dt.float32, kind="ExternalOutput").ap()

    from concourse.tile import TileContext

    with TileContext(nc) as tc, ExitStack() as ctx:
        const = ctx.enter_context(tc.tile_pool(name="const", bufs=1))
        pers = ctx.enter_context(tc.tile_pool(name="pers", bufs=1))
        dbl = ctx.enter_context(tc.tile_pool(name="dbl", bufs=2))
        ohp = ctx.enter_context(tc.tile_pool(name="ohp", bufs=3))
        ohc = ctx.enter_context(tc.tile_pool(name="ohc", bufs=3))
        stpp = ctx.enter_context(tc.tile_pool(name="stpp", bufs=3))
        pp = ctx.enter_context(tc.tile_pool(name="pp", bufs=4, space="PSUM"))
        pt = ctx.enter_context(tc.tile_pool(name="pt", bufs=2, space="PSUM"))
        pz = ctx.enter_context(tc.tile_pool(name="pz", bufs=2, space="PSUM"))

        # ---- persistent constants ----
        tWq = const.tile([128, 6 * D], dt.bfloat16)
        tWk = const.tile([128, 6 * D], dt.bfloat16)
        tWv = const.tile([128, 6 * D], dt.bfloat16)
        tWo = const.tile([128, 6 * D], dt.bfloat16)
        for t, d_ in ((tWq, Wq_d), (tWk, Wk_d), (tWv, Wv_d), (tWo, Wo_d)):
            nc.sync.dma_start(t[:].rearrange("p (c o) -> p c o", c=6),
                              d_.rearrange("(c p) o -> p c o", p=128))
        tbqs = const.tile([128, 6], dt.float32)
        nc.sync.dma_start(tbqs[:], bqs_d.rearrange("(c p) -> p c", p=128))
        tbk = const.tile([128, 6], dt.float32)
        nc.sync.dma_start(tbk[:], bk_d.rearrange("(c p) -> p c", p=128))
        tbvb = const.tile([1, D], dt.bfloat16); nc.sync.dma_start(tbvb[:], bvb_d[:])
        tbob = const.tile([1, D], dt.bfloat16); nc.sync.dma_start(tbob[:], bob_d[:])
        tEq = const.tile([128, H * TC], dt.bfloat16)
        tEk = const.tile([128, H * TC], dt.bfloat16)
        for base in (0, 64):
            nc.sync.dma_start(tEq[base:base + 64, :].rearrange("d (h t) -> d h t", h=H),
                              Eq_d.rearrange("h d t -> d h t"))
            nc.sync.dma_start(tEk[base:base + 64, :].rearrange("d (h t) -> d h t", h=H),
                              Ek_d.rearrange("h d t -> d h t"))
        tWc = const.tile([TC, H * RD], dt.bfloat16)
        nc.sync.dma_start(tWc[:].rearrange("t (h d) -> t h d", h=H),
                          Wc_d.rearrange("h t d -> t h d"))
        tones = const.tile([1, 128], dt.bfloat16)
        nc.sync.dma_start(tones[:], ones_d[:])
        onesq = const.tile([128, 128], dt.bfloat16)
        nc.vector.memset(onesq[:], 1.0)
        ident = const.tile([128, 128], dt.bfloat16)
        nc.gpsimd.affine_select(ident[:], onesq[:], [[1, 128]], ALU.is_equal,
                                0.0, base=0, channel_multiplier=-1)
        identf = const.tile([128, 128], dt.float32)
        nc.vector.tensor_copy(identf[:], ident[:])

        # ---- persistent (single-buffered) per-batch workspace ----
        tET = [pers.tile([128, H * N], dt.bfloat16, name=f"tET{c_}") for c_ in range(2)]
        tAeT = pers.tile([TC, H * N], dt.bfloat16)
        tZ = pers.tile([128, 2 * H], dt.float32)
        tZr = pers.tile([128, 2 * H], dt.float32)
        tzT = pers.tile([128, 6 * N], dt.bfloat16)
        ty = pers.tile([128, D], dt.float32)
        tBqT = [pers.tile([128, H * N], dt.float32, name=f"tBqT{c_}")
                for c_ in range(2)]

        def qslab(t, h):
            # [64, 256] head-h slab of a [dout-part, (chunk, tok)] projection
            return t[(h % 2) * 64:(h % 2) * 64 + 64, (h // 2) * N:(h // 2) * N + N]

        def body(bi):
            # ---------- load xT ----------
            txT = dbl.tile([128, 6 * N], dt.bfloat16, tag="txT")
            nc.sync.dma_start(txT[:].rearrange("p (c n) -> p c n", c=6),
                              xT_d[bi].rearrange("(c p) n -> p c n", p=128))

            # ---------- projections ----------
            tq = dbl.tile([128, 6 * N], dt.bfloat16, tag="tq")
            tk = dbl.tile([128, 6 * N], dt.bfloat16, tag="tk")
            tv = dbl.tile([128, 2 * D], dt.bfloat16, tag="tv")
            for dc in range(6):
                for tW, tdst, scale, tbias in ((tWq, tq, S, tbqs), (tWk, tk, 1.0, tbk)):
                    ps = pp.tile([128, 512], dt.float32, tag="ps", name="ps_qk")
                    for dn in range(6):
                        nc.tensor.matmul(ps[:, 0:N],
                                         tW[:, dn * D + dc * 128: dn * D + dc * 128 + 128],
                                         txT[:, dn * N: dn * N + N],
                                         start=(dn == 0), stop=(dn == 5))
                    nc.scalar.activation(tdst[:, dc * N:dc * N + N], ps[:, 0:N], AF.Identity,
                                         bias=tbias[:, dc:dc + 1], scale=scale)
            for tcn in range(2):
                for half in range(2):
                    ps = pp.tile([128, 512], dt.float32, tag="ps", name="ps_v")
                    for dn in range(6):
                        nc.tensor.matmul(ps[:, 0:384],
                                         txT[:, dn * N + tcn * 128: dn * N + tcn * 128 + 128],
                                         tWv[:, dn * D + half * 384: dn * D + half * 384 + 384],
                                         start=(dn == 0), stop=False)
                    nc.tensor.matmul(ps[:, 0:384], tones[:], tbvb[:, half * 384: half * 384 + 384],
                                     start=False, stop=True)
                    nc.vector.tensor_copy(tv[:, tcn * D + half * 384: tcn * D + half * 384 + 384],
                                          ps[:, 0:384])

            # ---------- score tables (h-major: col = h*N + i) ----------
            tTq = dbl.tile([TC, H * N], dt.bfloat16, tag="tTq")
            tTk = dbl.tile([TC, H * N], dt.bfloat16, tag="tTk")
            for h in range(H):
                base = (h % 2) * 64
                for tE_, tsrc, tdst in ((tEq, tq, tTq), (tEk, tk, tTk)):
                    ps = pp.tile([128, 512], dt.float32, tag="ps", name="ps_tab")
                    nc.tensor.matmul(ps[0:TC, 0:N],
                                     tE_[base:base + 64, h * TC: h * TC + TC],
                                     qslab(tsrc, h),
                                     start=True, stop=True)
                    nc.scalar.activation(tdst[:, h * N: h * N + N],
                                         ps[0:TC, 0:N], AF.Copy)

            # ---------- bias matmuls (q-side then k-side) ----------
            # Both sides stage [ (c,h), (g,*) ] then PE-transpose out of the
            # awkward orientation. k-side -> tBk[i, (h,j)] bf16 (ident-add);
            # q-side -> tBqT[j, (h,i)] f32 (is_transpose-add into psA).
            tBk = [pers.tile([128, H * N], dt.bfloat16, tag=f"tBk{c_}", name=f"tBk{c_}")
                   for c_ in range(2)]
            for side in range(2):
                O_d, tT = (OA_d, tTq) if side == 0 else (OB_d, tTk)

                def bias_mm(ch, O_d=O_d, tT=tT):
                    i0 = ch * IC
                    tO = ohp.tile([TC, IC * N], dt.float8e4, tag="oh", name="tO")
                    nc.sync.dma_start(tO[:].rearrange("t (i j) -> t i j", i=IC),
                                      O_d[bi, :, i0:i0 + IC, :])
                    stg = dbl.tile([128, 8 * N], dt.bfloat16, tag="stq")
                    pbufs = [pp.tile([128, 512], dt.float32, tag="ps", name=f"psb{q_}")
                             for q_ in range(2)]
                    tTv = tT[:].rearrange("t (h n) -> t n h", n=N)
                    for g in range(8):
                        ps = pbufs[(g // 2) % 2]
                        col = (g % 2) * N
                        for c in range(4):
                            irel = c * 8 + g
                            nc.tensor.matmul(
                                ps[32 * c:32 * c + 12, col:col + N],
                                tTv[:, i0 + irel, :],
                                tO[:, irel * N: irel * N + N],
                                start=True, stop=True, tile_position=(0, 32 * c))
                        if g % 2 == 1:
                            if (g // 2) % 2 == 0:
                                nc.scalar.activation(stg[:, (g - 1) * N:(g + 1) * N], ps[:], AF.Copy)
                            else:
                                nc.vector.tensor_copy(stg[:, (g - 1) * N:(g + 1) * N], ps[:])
                    return stg

                # PE-transpose [ (c,h), (g,x) ] -> [ x, (c,h) ] and
                # assemble the bias buffer with strided copies.
                # side 0: x=j (chunk over i), dst tBqT[j, (h, i)] f32.
                # side 1: x=i (chunk over j), dst tBk[i, (h, j)] bf16.
                def bias_tr(ch, stg, side=side):
                    for xhalf in range(2):
                        if side == 0:
                            dstv = tBqT[xhalf][:].rearrange(
                                "p (h ich c g) -> p ich g c h",
                                h=H, ich=NCH, c=4)
                        else:
                            dstv = tBk[xhalf][:].rearrange(
                                "p (h jch c g) -> p jch g c h",
                                h=H, jch=NCH, c=4)
                        for gb in range(2):
                            pst = pt.tile([128, 512], dt.bfloat16,
                                          tag="pstr", name="pstb")
                            for gg in range(4):
                                g = gb * 4 + gg
                                nc.tensor.matmul(
                                    pst[:, gg * 128: gg * 128 + 128],
                                    stg[:, g * N + xhalf * 128:
                                        g * N + xhalf * 128 + 128],
                                    ident[:], is_transpose=True,
                                    start=True, stop=True)
                            srcv = pst[:].rearrange(
                                "p (gg c h2) -> p gg c h2", gg=4, c=4)
                            if (gb + xhalf) % 2 == 0:
                                nc.vector.tensor_copy(
                                    dstv[:, ch, gb * 4:(gb + 1) * 4],
                                    srcv[:, :, :, 0:H])
                            else:
                                nc.scalar.activation(
                                    dstv[:, ch, gb * 4:(gb + 1) * 4],
                                    srcv[:, :, :, 0:H], AF.Copy)

                prev_stg = None
                for ch in range(NCH):
                    stg_c = bias_mm(ch)
                    if prev_stg is not None:
                        bias_tr(ch - 1, prev_stg)
                    prev_stg = stg_c
                bias_tr(NCH - 1, prev_stg)

            # ---------- attention + per-head softmax/transpose ----------
            for h in range(H):
                psA = pp.tile([128, 512], dt.float32, tag="ps", name="psA")
                tEa = dbl.tile([128, 512], dt.bfloat16, tag="tEa")
                for icx in range(2):
                    acol = icx * N
                    nc.tensor.matmul(psA[:, acol:acol + N],
                                     qslab(tq, h)[:, icx * 128: icx * 128 + 128],
                                     qslab(tk, h),
                                     start=True, stop=False)
                    nc.tensor.matmul(psA[:, acol:acol + N], ident[:],
                                     tBk[icx][:, h * N: h * N + N],
                                     start=False, stop=False)
                    for jc in range(2):
                        nc.tensor.matmul(
                            psA[:, acol + jc * 128: acol + jc * 128 + 128],
                            tBqT[jc][:, h * N + icx * 128: h * N + icx * 128 + 128],
                            identf[:],
                            is_transpose=True, start=False, stop=(jc == 1))
                    nc.scalar.activation(
                        tEa[:, acol:acol + N],
                        psA[:, acol:acol + N], AF.Exp,
                        accum_out=tZ[:, 2 * h + icx: 2 * h + icx + 1])
                nc.vector.reciprocal(tZr[:, 2 * h:2 * h + 2], tZ[:, 2 * h:2 * h + 2])
                for icx in range(2):
                    nc.vector.tensor_scalar(
                        tEa[:, icx * N:icx * N + N],
                        tEa[:, icx * N:icx * N + N],
                        tZr[:, 2 * h + icx: 2 * h + icx + 1], None, ALU.mult)
                for jc in range(2):
                    pst = pt.tile([128, 512], dt.bfloat16, tag="pstr", name="pst")
                    for icx in range(2):
                        nc.tensor.matmul(pst[:, icx * 128: icx * 128 + 128],
                                         tEa[:, icx * N + jc * 128:
                                             icx * N + jc * 128 + 128],
                                         ident[:], is_transpose=True,
                                         start=True, stop=True)
                    nc.vector.tensor_copy(tET[jc][:, h * N: h * N + N],
                                          pst[:, 0:N])

            # ---------- pooling matmuls ----------
            def pool_mm(ch):
                i0 = ch * IC
                tOC = [ohc.tile([128, IC * TC], dt.float8e4, tag="ohc", name=f"tOC{jc}")
                       for jc in range(2)]
                for jc in range(2):
                    nc.sync.dma_start(tOC[jc][:].rearrange("j (i t) -> j i t", i=IC),
                                      OC_d[bi, jc * 128:jc * 128 + 128, i0:i0 + IC, :])
                stp = stpp.tile([128, 8 * TC], dt.bfloat16, tag="stp")
                pbufs = [pp.tile([128, 512], dt.float32, tag="ps", name=f"psp{q_}")
                         for q_ in range(2)]
                tETv = [tET[jc][:].rearrange("j (h n) -> j n h", n=N)
                        for jc in range(2)]
                for g in range(8):
                    ps = pbufs[(g // 4) % 2]
                    col = (g % 4) * TC
                    for c in range(4):
                        irel = c * 8 + g
                        for jc in range(2):
                            nc.tensor.matmul(
                                ps[32 * c:32 * c + 12, col:col + TC],
                                tETv[jc][:, i0 + irel, :],
                                tOC[jc][:, irel * TC: irel * TC + TC],
                                start=(jc == 0), stop=(jc == 1),
                                tile_position=(0, 32 * c))
                    if g % 4 == 3:
                        if (g // 4) % 2 == 0:
                            nc.scalar.activation(stp[:, (g - 3) * TC:(g + 1) * TC],
                                                 ps[:, 0:4 * TC], AF.Copy)
                        else:
                            nc.vector.tensor_copy(stp[:, (g - 3) * TC:(g + 1) * TC],
                                                  ps[:, 0:4 * TC])
                return stp

            # PE-transpose [ (c,h), (g,t) ] -> [ t, (c,h) ] and assemble
            # tAeT[t, (h, i0+c*8+g)] with strided copies.
            def pool_tr(ch, stp):
                dstpv = tAeT[:].rearrange("t (h ich c g) -> t ich g c h",
                                          h=H, ich=NCH, c=4)
                for gb in range(2):
                    pst = pt.tile([128, 512], dt.bfloat16, tag="pstr", name="pstp")
                    for gg in range(4):
                        g = gb * 4 + gg
                        nc.tensor.matmul(pst[0:TC, gg * 128: gg * 128 + 128],
                                         stp[:, g * TC: g * TC + TC],
                                         ident[:], is_transpose=True,
                                         start=True, stop=True)
                    srcpv = pst[0:TC, :].rearrange("t (gg c h2) -> t gg c h2",
                                                   gg=4, c=4)
                    if gb == 0:
                        nc.vector.tensor_copy(dstpv[:, ch, gb * 4:(gb + 1) * 4],
                                              srcpv[:, :, :, 0:H])
                    else:
                        nc.scalar.activation(dstpv[:, ch, gb * 4:(gb + 1) * 4],
                                             srcpv[:, :, :, 0:H], AF.Copy)

            prev_stp = None
            for ch in range(NCH):
                stp_c = pool_mm(ch)
                if prev_stp is not None:
                    pool_tr(ch - 1, prev_stp)
                prev_stp = stp_c
            pool_tr(NCH - 1, prev_stp)

            # ---------- AV + pooled values -> z^T ----------
            for h in range(H):
                pzt = pz.tile([128, 512], dt.float32, tag="pz", name="pz")
                for jc in range(2):
                    nc.tensor.matmul(pzt[0:64, 0:N],
                                     tv[:, jc * D + h * RD: jc * D + h * RD + RD],
                                     tET[jc][:, h * N: h * N + N],
                                     start=(jc == 0), stop=False)
                nc.tensor.matmul(pzt[0:64, 0:N], tWc[:, h * RD: h * RD + RD],
                                 tAeT[:, h * N: h * N + N],
                                 start=False, stop=True)
                nc.scalar.activation(qslab(tzT, h), pzt[0:64, 0:N], AF.Copy)

            # ---------- output projection ----------
            for icx in range(2):
                for half in range(2):
                    ps = pp.tile([128, 512], dt.float32, tag="ps", name="psy")
                    for dzc in range(6):
                        nc.tensor.matmul(ps[:, 0:384],
                                         tzT[:, dzc * N + icx * 128: dzc * N + icx * 128 + 128],
                                         tWo[:, dzc * D + half * 384: dzc * D + half * 384 + 384],
                                         start=(dzc == 0), stop=False)
                    nc.tensor.matmul(ps[:, 0:384], tones[:], tbob[:, half * 384: half * 384 + 384],
                                     start=False, stop=True)
                    nc.vector.tensor_copy(ty[:, half * 384: half * 384 + 384], ps[:, 0:384])
                nc.sync.dma_start(y_d[bi][icx * 128: icx * 128 + 128, :], ty[:])

        for b in range(nb):
            body(b)

    nc.compile()
    return nc


# ---------------------------------------------------------------- entry point
_PROGRAM_CACHE = {}


def _get_program(nb, ncores):
    key = (nb, ncores)
    if key not in _PROGRAM_CACHE:
        _PROGRAM_CACHE[key] = build_program(nb, num_devices=ncores)
    return _PROGRAM_CACHE[key]


def kernel(node_reps, connection_reps, distance, mask,
           Wq, bq, Wk, bk, Wv, bv, Wo, bo,
           Eeq, Eek, Eev, Epq, Epk, Epv):
    """Full-input GRPE attention on 8 TRN2 NeuronCores (data-parallel over batch)."""
    import antenv
    if '/opt/trn_rl_repo/antenv' not in antenv.__path__:
        antenv.__path__.append('/opt/trn_rl_repo/antenv')
    try:
        import antenv.axon_hooks as axon_hooks
        axon_hooks.register_default_hook()
    except Exception:
        pass
    from concourse.bass_utils import run_bass_kernel_spmd

    node_reps = np.asarray(node_reps)
    connection_reps = np.asarray(connection_reps)
    distance = np.asarray(distance)
    B = node_reps.shape[0]
    NCORES = 8
    assert B % NCORES == 0
    nb = B // NCORES

    inp = dict(Wq=np.asarray(Wq), bq=np.asarray(bq), Wk=np.asarray(Wk),
               bk=np.asarray(bk), Wv=np.asarray(Wv), bv=np.asarray(bv),
               Wo=np.asarray(Wo), bo=np.asarray(bo),
               Eeq=np.asarray(Eeq), Eek=np.asarray(Eek), Eev=np.asarray(Eev),
               Epq=np.asarray(Epq), Epk=np.asarray(Epk), Epv=np.asarray(Epv))
    w = prep_weights(inp)
    shards = [prep_shard(node_reps, connection_reps, distance, c * nb, nb)
              for c in range(NCORES)]

    nc = _get_program(nb, NCORES)
    in_maps = [{**w, **shards[c]} for c in range(NCORES)]
    res = run_bass_kernel_spmd(nc, in_maps, list(range(NCORES)))
    out = np.concatenate([res.results[c]["y"] for c in range(NCORES)], axis=0)
    return out.astype(np.float32)

